# revision 95
# baseline (speedup 1.0000x reference)
"""AttentionBlock (GroupNorm + single-head 4096x4096 attention + out-proj) on 8
Trainium2 NeuronCores, data-parallel over batch (B=8 -> 1 image per core).

Numerics: scores q^T k must be fp32-grade (softmax over scores with sigma~2500
behaves like an argmax; bf16/fp32r-level score error flips argmaxes and fails).
Scores = main fp16 term qh^T hnh (exact products, fp32 PSUM accumulation) plus
BOTH hi/lo cross terms as fp8e4m3 DoubleRow matmuls at 0.5 cyc/row with
product-preserving scales (ql*1 x hnh*1, qh/512 x hnl*512), K=256 packed into
128 partitions. The post-softmax path (e, v, AV matmul) is bf16.

Schedule: per 128-row i-tile the 4096 scores are produced in 4 PSUM chunks of
1024; each chunk gets a DVE max then an ACT exp straight from PSUM with the
chunk-local max as bias (accum_out = chunk denominator), freeing the PSUM bank
immediately. Normalization is deferred: a short combine computes per-chunk
factors exp(cmax-m)/den, applied chunk-wise on DVE/Pool right before batched
1024-column DMA-xbar transposes. Each tile's combine/normalize/transpose chain
is emitted after the NEXT tile's first chunk matmuls so it never head-blocks
the DVE/ACT queues. The AV matmul for a block of 4 i-tiles is software-
pipelined two tiles behind the stripes it consumes, split over four tiles
(cols 0:256 / 256:384 / 384:448 / 448:512) with each tile's share further
interleaved a quarter at a time between its score chunks, stretching PE's
time per chunk so the max->exp->PSUM-release chain always keeps up. The q-projection (u = M2^T hn,
fp16 hi/lo pieces) is emitted kc-major for its first column chunk in the
pre-phase and deferred into loop tiles 5..22 for the rest; the v-projection is
interleaved between score tile 0's chunks as PE filler while the GroupNorm/
cast pipeline produces the later j-chunks.
"""
import sys
if "/opt/trn_rl_repo" not in sys.path:
    sys.path.insert(0, "/opt/trn_rl_repo")

from contextlib import ExitStack

import numpy as np

import concourse.bacc as bacc
import concourse.tile as tile
from concourse import mybir
from concourse.bass_utils import run_bass_kernel_spmd

B, C, H, W = 8, 256, 64, 64
HW = H * W            # 4096
G = 32                # groups
GS = C // G           # 8 channels / group
EPS = 1e-5
NT = HW // 128        # 32 i-tiles
NB = NT // 2          # 16 AV blocks of 256 columns

f32 = mybir.dt.float32
f8e4 = mybir.dt.float8e4
f16 = mybir.dt.float16
bf16 = mybir.dt.bfloat16

_PROGRAMS = {}


def _build_program(qk_bias, v_bias, fp8_mode=2):
    # fp8_mode: 0 = all-fp16 scores (safest), 1 = ql*hnh cross in fp8 DR,
    # 2 = both crosses (ql*hnh and qh*hnl) in fp8 DR (fastest; rel~1e-2)
    fp8_cross = fp8_mode >= 1
    fp8_cross2 = fp8_mode >= 2
    # product-preserving fp8 scales for the qh*hnl cross term
    C2_SH = 512.0
    nc = bacc.Bacc("TRN2", target_bir_lowering=False, debug=False)

    def din(name, shape, dt=f32):
        return nc.dram_tensor(name, shape, dt, kind="ExternalInput").ap()

    d_x = din("x", (C, HW))
    # weights pre-transposed to [cin, cout]; q side pre-scaled by -0.5*C
    if qk_bias:
        d_wq_h, d_wq_l = din("wq_h", (C, C), f16), din("wq_l", (C, C), f16)
        d_wk_h, d_wk_l = din("wk_h", (C, C), f16), din("wk_l", (C, C), f16)
    else:
        # zero q/k bias: scores fold to u^T hn with u = M2^T hn, M2 = Wq'^T Wk
        d_m2_h, d_m2_l = din("m2_h", (C, C), f16), din("m2_l", (C, C), f16)
    d_wv_h = din("wv_h", (C, C), f16)
    d_wo_h = din("wo_h", (C, C), f16)
    d_gamma, d_beta = din("gamma", (C,)), din("beta", (C,))
    d_bq = din("bq", (1, 2, C), f16)   # pre-scaled by -0.5*C, [hi, lo]
    d_bk = din("bk", (1, 2, C), f16)   # [hi, lo]
    d_bv = din("bv", (1, C), f16)
    d_bo = din("bo", (C,))
    d_gmat = din("gmat", (128, 128))  # block-diag 1/GS group-averaging matrix
    d_out = nc.dram_tensor("out", (C, HW), f32, kind="ExternalOutput").ap()

    with tile.TileContext(nc) as tc, ExitStack() as ctx:
        main = ctx.enter_context(tc.tile_pool(name="main", bufs=1))
        hold = ctx.enter_context(tc.tile_pool(name="hold", bufs=1))
        psA = ctx.enter_context(tc.tile_pool(name="psA", bufs=3, space="PSUM"))
        psB = ctx.enter_context(tc.tile_pool(name="psB", bufs=2, space="PSUM"))

        # ---------------- persistent tiles ----------------
        wv_h = main.tile([128, 2, C], f16, name="wv_h")
        wo_h = main.tile([128, 2, C], f16, name="wo_h")
        # wo_h first and via SWDGE: it feeds the very first PE matmuls and
        # must not queue behind the x-chunk loads on the HWDGE queues
        nc.gpsimd.dma_start(wo_h, d_wo_h.rearrange("(kc kl) m -> kl kc m", kl=128))
        if qk_bias:
            wq_h = main.tile([128, 2, C], f16, name="wq_h")
            wq_l = main.tile([128, 2, C], f16, name="wq_l")
            wk_h = main.tile([128, 2, C], f16, name="wk_h")
            wk_l = main.tile([128, 2, C], f16, name="wk_l")
            wloads = [(wk_h, d_wk_h), (wk_l, d_wk_l), (wq_h, d_wq_h),
                      (wq_l, d_wq_l), (wv_h, d_wv_h)]
        else:
            m2_h = main.tile([128, 2, C], f16, name="m2_h")
            m2_l = main.tile([128, 2, C], f16, name="m2_l")
            wloads = [(m2_h, d_m2_h), (m2_l, d_m2_l), (wv_h, d_wv_h)]
        _wloads_todo = wloads

        # score-matmul operand pairs: (q,k) hi/lo, or (u, hn) hi/lo
        qh = main.tile([128, 2, HW], f16, name="qh")
        ql = None if fp8_cross else main.tile([128, 2, HW], f16, name="ql")
        ul8 = hh8 = qh8 = hl8 = None
        if fp8_cross:
            ul8 = main.tile([128, 2, HW], f8e4, name="ul8")
            hh8 = main.tile([128, 2, HW], f8e4, name="hh8")
        if fp8_cross2:
            qh8 = main.tile([128, 2, HW], f8e4, name="qh8")
            hl8 = main.tile([128, 2, HW], f8e4, name="hl8")
        if qk_bias:
            kh = main.tile([128, 2, HW], f16, name="kh")
            kl = main.tile([128, 2, HW], f16, name="kl")
            hn_pool = None
        else:
            hn_pool = main   # hn hi/lo persists as the scores rhs
        vT = main.tile([128, NT, C], bf16, name="vT")

        gmat = main.tile([128, 128], f32, name="gmat")
        nc.sync.dma_start(gmat, d_gmat)
        gamma_sb = main.tile([128, 2], f32, name="gamma_sb")
        beta_sb = main.tile([128, 2], f32, name="beta_sb")
        bo_sb = main.tile([128, 2], f32, name="bo_sb")
        nc.sync.dma_start(gamma_sb, d_gamma.rearrange("(t p) -> p t", p=128))
        nc.sync.dma_start(beta_sb, d_beta.rearrange("(t p) -> p t", p=128))
        nc.sync.dma_start(bo_sb, d_bo.rearrange("(t p) -> p t", p=128))
        bq_row = bk_row = bv_row = ones_row = None
        if qk_bias:
            bq_row = main.tile([1, 2, C], f16, name="bq_row")
            bk_row = main.tile([1, 2, C], f16, name="bk_row")
            nc.sync.dma_start(bq_row, d_bq)
            nc.sync.dma_start(bk_row, d_bk)
        if v_bias:
            bv_row = main.tile([1, C], f16, name="bv_row")
            nc.sync.dma_start(bv_row, d_bv)
        if qk_bias or v_bias:
            ones_row = main.tile([1, 512], f16, name="ones_row")
            nc.vector.memset(ones_row, 1.0)
        eps_t = main.tile([128, 2], f32, name="eps_t")
        nc.vector.memset(eps_t, EPS)

        proj_sb = main.tile([128, 2, HW], f16, name="proj_sb")

        with tc.tile_pool(name="pre", bufs=1) as pre:
            # -------------- load x, GroupNorm stats --------------
            # chunked loads/casts so downstream work starts on chunk 0 early
            x_sb = pre.tile([128, 2, HW], f32, name="x_sb")
            d_xv = d_x.rearrange("(t p) n -> p t n", p=128)
            # t-major so the t=0 GroupNorm stats (and the t=0 hn chunks the
            # projections consume first) are unblocked by the first 8 loads
            for t in range(2):
                for c8 in range(8):
                    n0 = c8 * 512
                    # first pair via the ACT HWDGE queues: they gate the first
                    # xh cast and all PE work, so skip the SP queue pile-up
                    eng = nc.scalar if (c8 == 0 and t == 0) else nc.sync
                    eng.dma_start(x_sb[:, t, n0:n0 + 512],
                                  d_xv[:, t, n0:n0 + 512])
                if t == 0:
                    # weight / small-vector loads queue AFTER the t=0 x chunks:
                    # nothing needs them before the stats chain completes
                    for wt, wd in _wloads_todo:
                        nc.sync.dma_start(
                            wt, wd.rearrange("(kc kl) m -> kl kc m", kl=128))

            xh = pre.tile([128, 2, HW], f16, name="xh")
            for c8 in range(8):
                for t in range(2):
                    n0 = c8 * 512
                    nc.scalar.activation(xh[:, t, n0:n0 + 512],
                                         x_sb[:, t, n0:n0 + 512],
                                         mybir.ActivationFunctionType.Copy)

            stats = pre.tile([128, 2, 8, 6], f32, name="stats")
            for t in range(2):
                xv = x_sb[:, t, :].rearrange("p (s n) -> p s n", n=512)
                for s in range(8):
                    nc.vector.bn_stats(stats[:, t, s, :], xv[:, s, :])
            # per-tile stats pipeline: tile 0's scale/bias is ready before
            # tile 1's stats finish, unblocking kc=0 projection matmuls early
            mv = pre.tile([128, 2, 2], f32, name="mv")
            a_sc = pre.tile([128, 2], f32, name="a_sc")
            b_sc = pre.tile([128, 2], f32, name="b_sc")
            stat2 = pre.tile([128, 2, 2], f32, name="stat2")
            gstat = pre.tile([128, 2, 2], f32, name="gstat")
            gvar = pre.tile([128, 2], f32, name="gvar")
            seps = pre.tile([128, 2], f32, name="seps")
            rstd = pre.tile([128, 2], f32, name="rstd")
            tmp = pre.tile([128, 2], f32, name="tmp")
            for t in range(2):
                nc.vector.bn_aggr(mv[:, t, :], stats[:, t, :, :])
                # stat2[:, t] = [mean_t, m2_t]
                nc.vector.tensor_tensor(out=stat2[:, t, 1:2], in0=mv[:, t, 0:1],
                                        in1=mv[:, t, 0:1], op=mybir.AluOpType.mult)
                nc.vector.tensor_tensor(out=stat2[:, t, 1:2], in0=stat2[:, t, 1:2],
                                        in1=mv[:, t, 1:2], op=mybir.AluOpType.add)
                nc.gpsimd.tensor_copy(stat2[:, t, 0:1], mv[:, t, 0:1])
                ps_g = psB.tile([128, 2], f32, name="ps_g", tag="psB")
                nc.tensor.matmul(ps_g, gmat, stat2[:, t, :], start=True, stop=True)
                nc.vector.tensor_copy(gstat[:, t, :], ps_g)
                gm = gstat[:, t, 0:1]
                nc.vector.tensor_tensor(out=gvar[:, t:t + 1], in0=gm, in1=gm,
                                        op=mybir.AluOpType.mult)
                nc.vector.tensor_tensor(out=gvar[:, t:t + 1], in0=gstat[:, t, 1:2],
                                        in1=gvar[:, t:t + 1],
                                        op=mybir.AluOpType.subtract)
                nc.vector.tensor_tensor(out=seps[:, t:t + 1], in0=gvar[:, t:t + 1],
                                        in1=eps_t[:, t:t + 1], op=mybir.AluOpType.add)
                nc.scalar.activation(rstd[:, t:t + 1], seps[:, t:t + 1],
                                     mybir.ActivationFunctionType.Sqrt)
                nc.vector.reciprocal(rstd[:, t:t + 1], rstd[:, t:t + 1])
                for _ in range(2):
                    nc.vector.tensor_tensor(out=tmp[:, t:t + 1], in0=rstd[:, t:t + 1],
                                            in1=rstd[:, t:t + 1], op=mybir.AluOpType.mult)
                    nc.vector.tensor_tensor(out=tmp[:, t:t + 1], in0=tmp[:, t:t + 1],
                                            in1=seps[:, t:t + 1], op=mybir.AluOpType.mult)
                    nc.vector.tensor_scalar(tmp[:, t:t + 1], tmp[:, t:t + 1], -0.5, 1.5,
                                            op0=mybir.AluOpType.mult,
                                            op1=mybir.AluOpType.add)
                    nc.vector.tensor_tensor(out=rstd[:, t:t + 1], in0=rstd[:, t:t + 1],
                                            in1=tmp[:, t:t + 1], op=mybir.AluOpType.mult)
                nc.vector.tensor_tensor(out=a_sc[:, t:t + 1], in0=rstd[:, t:t + 1],
                                        in1=gamma_sb[:, t:t + 1], op=mybir.AluOpType.mult)
                nc.vector.tensor_tensor(out=b_sc[:, t:t + 1], in0=gstat[:, t, 0:1],
                                        in1=a_sc[:, t:t + 1], op=mybir.AluOpType.mult)
                nc.vector.tensor_tensor(out=b_sc[:, t:t + 1], in0=beta_sb[:, t:t + 1],
                                        in1=b_sc[:, t:t + 1], op=mybir.AluOpType.subtract)

            # -------------- out-projection Wo@x + bo -> DRAM scratch ----------
            # (runs on PE while DVE computes hn; uses xh which is pre-norm x)
            def emit_wo(nch_lo, nch_hi):
                for nch in range(nch_lo, nch_hi):
                    n0 = nch * 512
                    for mc in range(2):
                        ps_p = psA.tile([128, 1024], f32, name="ps_p",
                                        tag="psA")
                        for kc in range(2):
                            nc.tensor.matmul(
                                ps_p[:, 0:512],
                                wo_h[:, kc, mc * 128:(mc + 1) * 128],
                                xh[:, kc, n0:n0 + 512],
                                start=(kc == 0), stop=(kc == 1))
                        if (nch + mc) % 2 == 0:
                            nc.vector.tensor_scalar(
                                proj_sb[:, mc, n0:n0 + 512], ps_p[:, 0:512],
                                bo_sb[:, mc:mc + 1], None,
                                op0=mybir.AluOpType.add)
                        else:
                            nc.scalar.activation(
                                proj_sb[:, mc, n0:n0 + 512], ps_p[:, 0:512],
                                mybir.ActivationFunctionType.Identity,
                                bias=bo_sb[:, mc:mc + 1])
            emit_wo(0, 4)

            # -------------- hn (in-place) and fp16 hi/lo split --------------
            # chunked so the q/k projections can start on early chunks
            hp = hn_pool if hn_pool is not None else pre
            hnh = hp.tile([128, 2, HW], f16, name="hnh")
            # hnl lives in `hold` (not `pre`): the nh>=1 projection steps are
            # deferred into the main loop and still consume it there
            hlp = pre if qk_bias else hold
            hnl = hlp.tile([128, 2, HW], f16, name="hnl")
            # c4-major so the first projection step's inputs (both kc planes
            # of column chunk 0) are produced first; hnh cast on ACT so the
            # DVE chunk chain is normalize+subtract only
            hn_order = [(0, 0), (1, 0), (2, 0), (0, 1),
                        (1, 1), (2, 1), (3, 0), (3, 1)]
            for c4, t in hn_order:
                if True:
                    n0 = c4 * 1024
                    sl = (slice(None), t, slice(n0, n0 + 1024))
                    nc.vector.tensor_scalar(x_sb[sl], x_sb[sl],
                                            a_sc[:, t:t + 1], b_sc[:, t:t + 1],
                                            op0=mybir.AluOpType.mult,
                                            op1=mybir.AluOpType.add)
                    nc.scalar.activation(hnh[sl], x_sb[sl],
                                         mybir.ActivationFunctionType.Copy)
                    nc.vector.tensor_tensor(out=hnl[sl], in0=x_sb[sl], in1=hnh[sl],
                                            op=mybir.AluOpType.subtract)
                    # fp8 conversions chunked at the production site so the
                    # first score tiles aren't gated on whole-tensor casts
                    if fp8_cross:
                        # scale-free (ql*1 x hnh*1 is product-preserving and
                        # numerically better; a Pool copy is also ~0.6us
                        # cheaper than a scaled multiply)
                        nc.gpsimd.tensor_copy(hh8[sl], hnh[sl])
                    if fp8_cross2:
                        nc.gpsimd.tensor_scalar(hl8[sl], hnl[sl], C2_SH, None,
                                                op0=mybir.AluOpType.mult)

            # -------------- q,k projections (fp16-split, exact-grade) --------
            if qk_bias:
                projs = [(wk_h, wk_l, bk_row, kh, kl),
                         (wq_h, wq_l, bq_row, qh, ql)]
            else:
                projs = [(m2_h, m2_l, None, qh, ql)]   # u = M2^T hn -> qh/ql
            # nh-major so both mc halves of the first column chunk land first,
            # unblocking score tile 0 after two proj steps
            def emit_proj_step(w_h, w_l, b_row, out_h, out_l, nh, mc,
                               scratch=None):
                n0 = nh * 1024
                ps_q = psA.tile([128, 1024], f32, name="ps_q", tag="psA")
                pieces = []
                for kc in range(2):
                    pieces += [
                        (w_h[:, kc, mc * 128:(mc + 1) * 128], hnh[:, kc]),
                        (w_l[:, kc, mc * 128:(mc + 1) * 128], hnh[:, kc]),
                        (w_h[:, kc, mc * 128:(mc + 1) * 128], hnl[:, kc]),
                    ]
                for idx, (lhs, rhsrow) in enumerate(pieces):
                    for ns in range(2):
                        j0 = n0 + ns * 512
                        nc.tensor.matmul(ps_q[:, ns * 512:(ns + 1) * 512],
                                         lhs, rhsrow[:, j0:j0 + 512],
                                         start=(idx == 0),
                                         stop=(not qk_bias and idx == len(pieces) - 1))
                if qk_bias:
                    for hl in range(2):
                        for ns in range(2):
                            nc.tensor.matmul(ps_q[:, ns * 512:(ns + 1) * 512],
                                             b_row[:, hl, mc * 128:(mc + 1) * 128],
                                             ones_row, start=False,
                                             stop=(hl == 1))
                nc.scalar.activation(out_h[:, mc, n0:n0 + 1024], ps_q,
                                     mybir.ActivationFunctionType.Copy)
                # with the ql*hnh cross in fp8, out_l is consumed only by the
                # ul8 cast: write it to a small rotating scratch instead of a
                # persistent [128,2,HW] tensor
                ol = scratch if scratch is not None \
                    else out_l[:, mc, n0:n0 + 1024]
                nc.vector.tensor_tensor(out=ol,
                                        in0=ps_q,
                                        in1=out_h[:, mc, n0:n0 + 1024],
                                        op=mybir.AluOpType.subtract)
                if out_h is qh:
                    # chunked fp8 casts right behind the producers; qh8 on
                    # ACT here (deferred steps run inside the loop, where
                    # DVE paces the softmax maxes)
                    if fp8_cross2:
                        nc.scalar.activation(
                            qh8[:, mc, n0:n0 + 1024],
                            out_h[:, mc, n0:n0 + 1024],
                            mybir.ActivationFunctionType.Copy,
                            scale=1.0 / C2_SH)
                    if fp8_cross:
                        nc.scalar.activation(
                            ul8[:, mc, n0:n0 + 1024],
                            ol,
                            mybir.ActivationFunctionType.Copy)

            # nh>=1 steps are deferred into the main loop (tiles 5..22) when
            # the m2 fast path is active; their outputs are consumed from
            # tile 8 on, so the pre-phase ACT/DVE load shrinks by 3/4
            def emit_proj_step_kcmajor(w_h, w_l, out_h, out_l, nh, scr,
                                       ps_qs=None, kcs=(0, 1), finish=True):
                # zero-bias path only: both mc halves' kc0 pieces stream
                # before any kc1 piece, so PE starts ~6us earlier (the kc1
                # hn plane depends on the second half of the x load)
                n0 = nh * 1024
                if ps_qs is None:
                    ps_qs = [psA.tile([128, 1024], f32, name=f"ps_q{m}",
                                      tag="psA") for m in range(2)]
                for kc in kcs:
                    for mc in range(2):
                        pieces = [
                            (w_h[:, kc, mc * 128:(mc + 1) * 128], hnh[:, kc]),
                            (w_l[:, kc, mc * 128:(mc + 1) * 128], hnh[:, kc]),
                            (w_h[:, kc, mc * 128:(mc + 1) * 128], hnl[:, kc]),
                        ]
                        for idx, (lhs, rhsrow) in enumerate(pieces):
                            for ns in range(2):
                                j0 = n0 + ns * 512
                                nc.tensor.matmul(
                                    ps_qs[mc][:, ns * 512:(ns + 1) * 512],
                                    lhs, rhsrow[:, j0:j0 + 512],
                                    start=(kc == 0 and idx == 0),
                                    stop=(kc == 1 and idx == 2))
                if not finish:
                    return ps_qs
                for mc in range(2):
                    ps_q = ps_qs[mc]
                    nc.scalar.activation(out_h[:, mc, n0:n0 + 1024], ps_q,
                                         mybir.ActivationFunctionType.Copy)
                    ol = scr[mc] if scr is not None \
                        else out_l[:, mc, n0:n0 + 1024]
                    nc.vector.tensor_tensor(out=ol, in0=ps_q,
                                            in1=out_h[:, mc, n0:n0 + 1024],
                                            op=mybir.AluOpType.subtract)
                    if fp8_cross2:
                        nc.vector.tensor_scalar(
                            qh8[:, mc, n0:n0 + 1024],
                            out_h[:, mc, n0:n0 + 1024],
                            1.0 / C2_SH, None, op0=mybir.AluOpType.mult)
                    if fp8_cross:
                        nc.scalar.activation(
                            ul8[:, mc, n0:n0 + 1024], ol,
                            mybir.ActivationFunctionType.Copy)

            defer_projs = not qk_bias
            if defer_projs:
                (w_h, w_l, b_row, out_h, out_l) = projs[0]
                scr = [pre.tile([128, 1024], f16, name=f"qls_p{m}")
                       for m in range(2)] if fp8_cross else None
                # kc0 pieces (need only the first x half) go out between the
                # wo-proj column batches; kc1 + epilogue after the rest
                qs = emit_proj_step_kcmajor(w_h, w_l, out_h, out_l, 0, scr,
                                            kcs=(0,), finish=False)
                emit_wo(4, 8)
                emit_proj_step_kcmajor(w_h, w_l, out_h, out_l, 0, scr,
                                       ps_qs=qs, kcs=(1,), finish=True)
            else:
                emit_wo(4, 8)
                for (w_h, w_l, b_row, out_h, out_l) in projs:
                    for nh in range(4):
                        for mc in range(2):
                            sc = pre.tile([128, 1024], f16,
                                          name=f"qls_p{nh}{mc}") \
                                if fp8_cross else None
                            emit_proj_step(w_h, w_l, b_row, out_h, out_l,
                                           nh, mc, scratch=sc)

            # -------------- v projection, transposed: vT[i, c] --------------
            def emit_vproj(it_lo, it_hi):
                # two i-tiles per PSUM tile -> one 512-wide copy instead of
                # two 256-wide ones (halves the copy instruction overhead in
                # the congested warmup window)
                for it2 in range(it_lo, it_hi, 2):
                    ps_v = psB.tile([128, 2, C], f32, name="ps_v", tag="psB")
                    for sub in range(2):
                        i0 = (it2 + sub) * 128
                        for kc in range(2):
                            nc.tensor.matmul(ps_v[:, sub, :],
                                             hnh[:, kc, i0:i0 + 128],
                                             wv_h[:, kc, :], start=(kc == 0),
                                             stop=(not v_bias and kc == 1))
                        if v_bias:
                            nc.tensor.matmul(ps_v[:, sub, :],
                                             ones_row[:, 0:128], bv_row,
                                             start=False, stop=True)
                    if it2 % 4 == 0:
                        nc.scalar.activation(vT[:, it2:it2 + 2, :], ps_v,
                                             mybir.ActivationFunctionType.Copy)
                    else:
                        nc.vector.tensor_copy(vT[:, it2:it2 + 2, :], ps_v)

            if not defer_projs:
                emit_vproj(0, NT)

        # ---------------- main attention loop ----------------
        # Per i-tile: scores in 4 PSUM chunks of 1024; each chunk gets a
        # DVE max (negated) then an ACT exp straight from PSUM with the
        # chunk-local max as bias (accum_out = chunk denominator), freeing
        # the PSUM bank immediately. Normalization is deferred: after the
        # last chunk a short combine computes per-chunk factors
        # s_jc = exp(cmax_jc - m) / den, applied chunk-wise (split DVE/ACT)
        # right before the (batched, 1024-col) DMA transposes.
        # The AV matmul for block b is software-pipelined one tile late in
        # three column stripes (0:256 after tile 4b+2, 256:384 after 4b+3,
        # 384:512 after 4b+4) so PE never waits on a softmax tail.
        with tc.tile_pool(name="loop", bufs=2) as lp:
            eT_cur = eT_prev = None
            ps_h = None
            Exp = mybir.ActivationFunctionType.Exp
            Copy = mybir.ActivationFunctionType.Copy

            def av_partial(eT_blk, c0, c1, start, stop):
                for jc in range(NT):
                    for mc in range(2):
                        nc.tensor.matmul(ps_h[mc][:, c0:c1],
                                         vT[:, jc, mc * 128:(mc + 1) * 128],
                                         eT_blk[:, jc, c0:c1],
                                         start=(start and jc == 0),
                                         stop=(stop and jc == NT - 1))

            def av_out_mc(blk_out, mc, ph):
                o_sb = lp.tile([128, 512], f32, name="o_sb", tag="o_sb",
                               bufs=3)
                nc.vector.tensor_tensor(out=o_sb, in0=ph[mc],
                                        in1=proj_sb[:, mc, blk_out * 512:(blk_out + 1) * 512],
                                        op=mybir.AluOpType.add)
                nc.sync.dma_start(
                    d_out[mc * 128:(mc + 1) * 128, blk_out * 512:(blk_out + 1) * 512],
                    o_sb)

            def av_out(blk_out):
                for mc in range(2):
                    av_out_mc(blk_out, mc, ps_h)

            pending = None

            for t in range(NT):
                i0 = t * 128
                blk, il = t // 4, t % 4
                if il == 0:
                    eT_prev = eT_cur
                    eT_cur = lp.tile([128, NT, 512], bf16, name="eT", tag="eT",
                                     bufs=2)

                # ---- scores + chunked softmax for i-tile t ----
                e_t = lp.tile([128, HW], bf16, name="e_t", tag="e_t", bufs=2)
                nm = lp.tile([128, 4], f32, name="nm", tag="nm")
                cden = lp.tile([128, 4], f32, name="cden", tag="cden")
                rh, rl = (kh, kl) if qk_bias else (hnh, hnl)
                pieces = []
                for kc in range(2):
                    pieces.append((qh[:, kc, i0:i0 + 128], rh[:, kc]))
                    if not fp8_cross:
                        pieces.append((ql[:, kc, i0:i0 + 128], rh[:, kc]))
                    if not fp8_cross2:
                        pieces.append((qh[:, kc, i0:i0 + 128], rl[:, kc]))
                dr_pairs = []
                if fp8_cross:
                    dr_pairs.append((ul8, hh8))
                if fp8_cross2:
                    dr_pairs.append((qh8, hl8))
                for jc in range(4):
                    ps_s = psA.tile([128, 1024], f32, name="ps_s", tag="psA")
                    for idx, (lhs, rhsrow) in enumerate(pieces):
                        for ns in range(2):
                            j0 = jc * 1024 + ns * 512
                            nc.tensor.matmul(ps_s[:, ns * 512:(ns + 1) * 512],
                                             lhs, rhsrow[:, j0:j0 + 512],
                                             start=(idx == 0),
                                             stop=(not dr_pairs and idx == len(pieces) - 1))
                    # cross-terms as fp8 DoubleRow matmuls (K=256 packed
                    # via the [Ki,2,dim] interleave; HW-probe-validated)
                    for di, (dl, drr) in enumerate(dr_pairs):
                        last = di == len(dr_pairs) - 1
                        for ns in range(2):
                            j0 = jc * 1024 + ns * 512
                            nc.tensor.matmul(ps_s[:, ns * 512:(ns + 1) * 512],
                                             dl[:, :, i0:i0 + 128],
                                             drr[:, :, j0:j0 + 512],
                                             start=False, stop=last,
                                             perf_mode=mybir.MatmulPerfMode.DoubleRow)
                    if jc == 0 and pending is not None:
                        # the previous tile's combine/normalize/transpose goes
                        # out BEFORE this tile's first max/exp: its inputs are
                        # already ready, so it streams on DVE/ACT while PE
                        # works through this chunk's matmuls
                        pending()
                        pending = None
                    # a quarter of this tile's AV partial rides between score
                    # chunks: it stretches PE's time per chunk so the
                    # max->exp->PSUM-release chain always keeps up
                    if t >= 3:
                        b0f, phasef = divmod(t - 3, 4)
                        eT_f = eT_cur if phasef == 0 else eT_prev
                        rng_f = [(0, 256), (256, 384), (384, 448),
                                 (448, 512)][phasef]
                        if phasef == 0 and jc == 0:
                            if b0f >= 1:
                                av_out_mc(b0f - 1, 1, ps_h)
                            ps_h = [psB.tile([128, 512], f32,
                                             name=f"ps_h{m}", tag="psB")
                                    for m in range(2)]
                        for jcc in range(jc * 8, (jc + 1) * 8):
                            for mc in range(2):
                                nc.tensor.matmul(
                                    ps_h[mc][:, rng_f[0]:rng_f[1]],
                                    vT[:, jcc, mc * 128:(mc + 1) * 128],
                                    eT_f[:, jcc, rng_f[0]:rng_f[1]],
                                    start=(jcc == 0),
                                    stop=(jcc == NT - 1))
                    # chunk max (negated) -> exp from PSUM, unnormalized;
                    # PSUM bank freed at exp end
                    nc.vector.tensor_reduce(nm[:, jc:jc + 1], ps_s,
                                            axis=mybir.AxisListType.X,
                                            op=mybir.AluOpType.max, negate=True)
                    nc.scalar.activation(e_t[:, jc * 1024:(jc + 1) * 1024],
                                         ps_s, Exp, bias=nm[:, jc:jc + 1],
                                         scale=1.0,
                                         accum_out=cden[:, jc:jc + 1])
                    if defer_projs and t == 0:
                        # PE filler while the hn/cast pipeline produces the
                        # next j-chunk (vproj batch jc needs only chunk jc)
                        emit_vproj(jc * 8, (jc + 1) * 8)

                def make_tail(t=t, il=il, e_t=e_t, nm=nm, cden=cden,
                              eT_dst=eT_cur):
                    def tail():
                        # combine: per-chunk factors exp(cmax_jc - m) / den
                        negm = lp.tile([128, 1], f32, name="negm", tag="negm")
                        fj = lp.tile([128, 4], f32, name="fj", tag="fj")
                        fc = lp.tile([128, 4], f32, name="fc", tag="fc")
                        den = lp.tile([128, 1], f32, name="den", tag="den")
                        rden = lp.tile([128, 1], f32, name="rden", tag="rden")
                        nc.vector.tensor_reduce(negm, nm,
                                                axis=mybir.AxisListType.X,
                                                op=mybir.AluOpType.min)
                        # fj = exp(-(nm - negm)) = exp(cmax - m)
                        nc.vector.tensor_scalar(fj, nm, negm, None,
                                                op0=mybir.AluOpType.subtract)
                        nc.scalar.activation(fj, fj, Exp, scale=-1.0)
                        # den = sum_jc fj*cden  (NOTE: tensor_tensor_reduce
                        # hard-crashes the device here - keep the 2-op form)
                        nc.vector.tensor_tensor(out=fc, in0=fj, in1=cden,
                                                op=mybir.AluOpType.mult)
                        nc.vector.tensor_reduce(den, fc,
                                                axis=mybir.AxisListType.X,
                                                op=mybir.AluOpType.add)
                        nc.vector.reciprocal(rden, den)
                        # normalize chunks with the fused (x*fj)*rden
                        # two-scalar form: chunks 0/3 on DVE, 1/2 on Pool
                        # (keeps ACT free for the exp stream, which paces
                        # PSUM release)
                        for jc in range(4):
                            sl = slice(jc * 1024, (jc + 1) * 1024)
                            eng = nc.vector if jc in (0, 3) else nc.gpsimd
                            eng.tensor_scalar(
                                e_t[:, sl], e_t[:, sl],
                                fj[:, jc:jc + 1], rden,
                                op0=mybir.AluOpType.mult,
                                op1=mybir.AluOpType.mult)
                            nc.sync.dma_start_transpose(
                                eT_dst[:, jc * 8:(jc + 1) * 8,
                                       il * 128:(il + 1) * 128],
                                e_t[:, sl])
                    return tail

                pending = make_tail()
                if t == NT - 1:
                    pending()
                    pending = None

                # ---- software-pipelined AV partials, two tiles behind the
                # stripe they consume so the transposes are always done:
                # t=4b+3: AV(b) cols 0:256; 4b+4: 256:384; 4b+5: 384:448;
                # 4b+6: 448:512 + output (the 4-way split keeps every tile
                # carrying PE work so DVE/ACT never fall behind)
                if t >= 3 and (t - 3) % 4 == 3:
                    av_out_mc((t - 3) // 4, 0, ps_h)
                if t == NT - 1:
                    # drain the final block's remaining stripes
                    av_partial(eT_cur, 256, 384, True, True)
                    av_partial(eT_cur, 384, 448, True, True)
                    av_partial(eT_cur, 448, 512, True, True)
                    av_out(NT // 4 - 1)


                if defer_projs and t in (5, 6, 13, 14, 21, 22):
                    dnh = 1 + (t - 5) // 8
                    dmc = (t - 5) % 8 % 2
                    (w_h, w_l, b_row, out_h, out_l) = projs[0]
                    sc = lp.tile([128, 1024], f16, name="qls",
                                 tag="qls") if fp8_cross else None
                    emit_proj_step(w_h, w_l, b_row, out_h, out_l, dnh, dmc,
                                   scratch=sc)

    _dedup_ldweights(nc)
    nc.compile()
    return nc


def _dedup_ldweights(nc):
    """Remove back-to-back InstLdweights that reload the identical stationary
    operand on the PE stream (tile splits every matmul into ldweights+matmult,
    even when consecutive matmuls share weights). Any sync info on a removed
    load is merged into the following kept PE instruction."""
    import concourse.mybir as mybir_m

    for f in nc.m.functions:
        for blk in f.blocks:
            insts = blk.instructions
            last_key = None
            pending_waits = []
            pending_updates = []
            keep = []
            removed = 0
            for inst in insts:
                tn = type(inst).__name__
                eng = str(inst.engine)
                if "PE" not in eng:
                    keep.append(inst)
                    continue
                if tn == "InstLdweights":
                    a = inst.ins[0]
                    key = (getattr(a, "memref", None), getattr(a, "offset", None),
                           str(getattr(a, "ap", None)), str(getattr(a, "dtype", None)))
                    if key == last_key:
                        si = inst.sync_info
                        if si is not None:
                            pending_waits += list(si.on_wait)
                            pending_updates += list(si.on_update)
                        removed += 1
                        continue
                    last_key = key
                elif tn not in ("InstMatmult",):
                    # unknown PE instruction: weights state no longer certain
                    last_key = None
                if (pending_waits or pending_updates):
                    si = inst.sync_info
                    if si is None:
                        inst.sync_info = mybir_m.SyncInfo(
                            on_wait=pending_waits, on_update=pending_updates)
                    else:
                        inst.sync_info = mybir_m.SyncInfo(
                            on_wait=list(si.on_wait) + pending_waits,
                            on_update=list(si.on_update) + pending_updates)
                    pending_waits, pending_updates = [], []
                keep.append(inst)
            if removed:
                while len(blk.instructions):
                    blk.instructions.pop()
                for inst in keep:
                    blk.instructions.append(inst)


def _get_program(qk_bias=True, v_bias=True, fp8_mode=2):
    key = (qk_bias, v_bias, fp8_mode)
    if key not in _PROGRAMS:
        _PROGRAMS[key] = _build_program(qk_bias, v_bias, fp8_mode)
    return _PROGRAMS[key]


def kernel(x, norm_gamma, norm_beta, Wq, bq, Wk, bk, Wv, bv, Wo, bo):
    x = np.ascontiguousarray(np.asarray(x, np.float32))
    assert x.shape == (B, C, H, W)

    def _bias_hl(b32):
        h = b32.astype(np.float16)
        l = (b32 - h.astype(np.float32)).astype(np.float16)
        return np.stack([h, l]).reshape(1, 2, C)

    def split16(w):
        h = w.astype(np.float16)
        l = (w - h.astype(np.float32)).astype(np.float16)
        return h, l

    scale = -0.5 * C
    wq_t = np.ascontiguousarray((np.asarray(Wq, np.float32) * scale).T)
    wk_t = np.ascontiguousarray(np.asarray(Wk, np.float32).T)
    wv_t = np.ascontiguousarray(np.asarray(Wv, np.float32).T)
    wo_t = np.ascontiguousarray(np.asarray(Wo, np.float32).T)
    wq_h, wq_l = split16(wq_t)
    wk_h, wk_l = split16(wk_t)
    # exact bilinear fold for the zero-bias fast path: s = hn^T M2 hn
    m2 = ((np.asarray(Wq, np.float64) * scale).T @ np.asarray(Wk, np.float64))
    m2_h = m2.astype(np.float16)
    m2_l = (m2 - m2_h.astype(np.float64)).astype(np.float16)
    wv_h = wv_t.astype(np.float16)
    wo_h = wo_t.astype(np.float16)

    gmat = np.zeros((128, 128), np.float32)
    for g in range(128 // GS):
        gmat[g * GS:(g + 1) * GS, g * GS:(g + 1) * GS] = 1.0 / GS

    common = {
        "wq_h": wq_h, "wq_l": wq_l, "wk_h": wk_h, "wk_l": wk_l,
        "wv_h": wv_h, "wo_h": wo_h,
        "m2_h": np.ascontiguousarray(m2_h), "m2_l": np.ascontiguousarray(m2_l),
        "gamma": np.asarray(norm_gamma, np.float32),
        "beta": np.asarray(norm_beta, np.float32),
        "bq": _bias_hl(np.asarray(bq, np.float32) * scale),
        "bk": _bias_hl(np.asarray(bk, np.float32)),
        "bv": np.asarray(bv, np.float32).astype(np.float16).reshape(1, C),
        "bo": np.asarray(bo, np.float32),
        "gmat": gmat,
    }
    in_maps = [dict(common, x=x[c].reshape(C, HW)) for c in range(B)]

    qk_bias = bool(np.any(np.asarray(bq)) or np.any(np.asarray(bk)))
    v_bias = bool(np.any(np.asarray(bv)))
    import os as _os
    fp8_mode = int(_os.environ.get("ATTN_FP8", "2")) if not qk_bias else 0
    nc = _get_program(qk_bias, v_bias, fp8_mode)
    global _LAST_PROGRAM
    _LAST_PROGRAM = nc
    import os
    trace = bool(os.environ.get("ATTN_TRACE"))
    res = run_bass_kernel_spmd(nc, in_maps, core_ids=list(range(B)),
                               trace=trace,
                               tmpdir=os.environ.get("ATTN_TRACE_DIR") or None)
    global _LAST_EXEC_NS
    _LAST_EXEC_NS = res.exec_time_ns
    out = np.stack([res.results[c]["out"].reshape(C, H, W) for c in range(B)])
    return out.astype(np.float32)


_LAST_EXEC_NS = None
_LAST_PROGRAM = None


if __name__ == "__main__":
    rng = np.random.default_rng(0)
    ins = {
        "x": rng.standard_normal((B, C, H, W)).astype(np.float32),
        "norm_gamma": np.ones(C, np.float32),
        "norm_beta": np.zeros(C, np.float32),
        "Wq": (rng.standard_normal((C, C)) / 16).astype(np.float32),
        "bq": np.zeros(C, np.float32),
        "Wk": (rng.standard_normal((C, C)) / 16).astype(np.float32),
        "bk": np.zeros(C, np.float32),
        "Wv": (rng.standard_normal((C, C)) / 16).astype(np.float32),
        "bv": np.zeros(C, np.float32),
        "Wo": (rng.standard_normal((C, C)) / 16).astype(np.float32),
        "bo": np.zeros(C, np.float32),
    }
    o = kernel(**ins)
    print("kernel ran, out shape", o.shape, "absmax", np.abs(o).max())

